# revision 31
# baseline (speedup 1.0000x reference)
"""Distributed Trainium2 kernel for nn_AltBlock (dense transformer block).

Sharding: sequence-parallel across 8 cores. Core c owns 256 query tokens of
batch c//4 (quarter (c%4) of the sequence). qkv/proj/mlp run per-core on the
local tokens with replicated weights; attention needs all keys/values of the
batch, obtained with two 512KB fp8 AllGathers (kn^T, then V) inside each
4-core batch group. Attention runs in the transposed layout
S^T = [k_tokens(part), q_tokens(free)]; alibi (with the padding mask baked
in) is added via identity-matmuls into PSUM; softmax row-sums accumulate
via a ones-matmul chain in a separate PSUM bank alongside the PV matmul.
proj and fc2 compute token-major (activations stationary, weights moving)
so the residual adds fuse into PSUM evacuation with no transposes.

Precision: Wqkv/Wproj fp8e4 (host-scaled x16), W2 fp8e3 (x128), W1 bf16,
activations bf16, LN affine folded into the consuming weight matrices on
the host. Measured rel err ~7.6e-3 vs the fp32 reference.
"""

import math
import numpy as np
from contextlib import ExitStack

B, N, C, H = 2, 1024, 1024, 16
D = C // H          # 64
HID = 4 * C         # 4096
NCORES = 8
GROUP = 4           # cores per batch
TLOC = N // GROUP   # 256 local (query) tokens per core
EPS = 1e-5
WSCALE = 16.0       # host multiplies weights by this; kernel divides on evac

_CACHE = {}
DEBUG = False


def _build_nc():
    import concourse.bass as bass
    import concourse.tile as tile
    from concourse import bacc, mybir

    f32 = mybir.dt.float32
    bf16 = mybir.dt.bfloat16
    f8 = mybir.dt.float8e4
    f8e3 = mybir.dt.float8e3
    AF = mybir.ActivationFunctionType
    OP = mybir.AluOpType
    RSCALE = 1.0 / WSCALE

    nc = bacc.Bacc(None, target_bir_lowering=False)

    x_in = nc.dram_tensor("x_loc", [TLOC, C], f32, kind="ExternalInput")
    alibi_in = nc.dram_tensor("alibi_t", [H, 8, 128, TLOC], bf16, kind="ExternalInput")
    wqkv_in = nc.dram_tensor("wqkv_t", [8, 128, 3 * C], f8, kind="ExternalInput")
    wproj_in = nc.dram_tensor("wproj_t", [8, 128, C], f8, kind="ExternalInput")
    w1_in = nc.dram_tensor("w1_t", [8, 128, HID], bf16, kind="ExternalInput")
    w2_in = nc.dram_tensor("w2_t", [32, 128, C], f8e3, kind="ExternalInput")
    bqkv_in = nc.dram_tensor("bqkv_t", [128, 24], f32, kind="ExternalInput")
    b1_in = nc.dram_tensor("b1_t", [128, 32], f32, kind="ExternalInput")
    bprow_in = nc.dram_tensor("bprow", [1, C], f32, kind="ExternalInput")
    b2row_in = nc.dram_tensor("b2row", [1, C], f32, kind="ExternalInput")
    scales_in = nc.dram_tensor("scales", [2, 8, 256], bf16, kind="ExternalInput")
    nbound_in = nc.dram_tensor("nbound", [128, H], f32, kind="ExternalInput")
    constsb_in = nc.dram_tensor("consts_bf", [128, 384], bf16, kind="ExternalInput")
    out_ext = nc.dram_tensor("out", [TLOC, C], f32, kind="ExternalOutput")
    if DEBUG:
        dbg_qkv = nc.dram_tensor("dbg_qkv", [128, 24 * TLOC], bf16, kind="ExternalOutput")
        dbg_kn = nc.dram_tensor("dbg_kn", [128, 8 * TLOC], f8, kind="ExternalOutput")
        dbg_qn = nc.dram_tensor("dbg_qn", [128, 8 * TLOC], bf16, kind="ExternalOutput")
        dbg_P = nc.dram_tensor("dbg_P", [128, 2 * 8 * TLOC], bf16, kind="ExternalOutput")
        dbg_OT = nc.dram_tensor("dbg_OT", [128, 8 * TLOC], bf16, kind="ExternalOutput")
        dbg_x1 = nc.dram_tensor("dbg_x1", [128, 2 * C], f32, kind="ExternalOutput")
        dbg_rs = nc.dram_tensor("dbg_rs", [64, TLOC], f32, kind="ExternalOutput")
        dbg_vr0 = nc.dram_tensor("dbg_vr0", [128, 4096], f8, kind="ExternalOutput")

    def bcast_ap(handle):
        ap = handle[:]
        return bass.AP(tensor=ap.tensor, offset=ap.offset, ap=[[0, 128], [1, C]])

    with ExitStack() as stack:
        stack.enter_context(nc.allow_low_precision(reason="bf16/fp8 compute"))
        tc = stack.enter_context(tile.TileContext(nc))
        pconst = stack.enter_context(tc.tile_pool(name="pconst", bufs=1))
        pdram = stack.enter_context(tc.tile_pool(name="pdram", bufs=1, space="DRAM"))

        # ---- inputs / persistents ----
        constsb_sb = pconst.tile([128, 384], bf16, name="constsb_sb")
        nc.sync.dma_start(constsb_sb, constsb_in[:])
        ident = constsb_sb[:, 0:128]
        sel_64 = constsb_sb[:, 192:194]
        ones64 = constsb_sb[:, 128:192]
        sel2T = constsb_sb[0:2, 194:322]

        bqkv_sb = pconst.tile([128, 24], f32, name="bqkv_sb")
        nc.sync.dma_start(bqkv_sb, bqkv_in[:])
        b1_sb = pconst.tile([128, 32], f32, name="b1_sb")
        nc.sync.dma_start(b1_sb, b1_in[:])
        bpbc_sb = pconst.tile([128, C], f32, name="bpbc_sb")
        nc.sync.dma_start(bpbc_sb, bcast_ap(bprow_in))
        b2bc_sb = pconst.tile([128, C], f32, name="b2bc_sb")
        nc.sync.dma_start(b2bc_sb, bcast_ap(b2row_in))
        scales_sb = pconst.tile([2, 8, 256], bf16, name="scales_sb")
        nc.sync.dma_start(scales_sb, scales_in[:])
        nbound_sb = pconst.tile([128, H], f32, name="nbound_sb")
        nc.sync.dma_start(nbound_sb, nbound_in[:])
        eps_sb = pconst.tile([128, 1], f32, name="eps_sb")
        nc.vector.memset(eps_sb, EPS)
        tiny_sb = pconst.tile([128, 1], f32, name="tiny_sb")
        nc.vector.memset(tiny_sb, 1e-24)

        # V bounce staging, [token, tt, head, 64] fp8. Softmax row-sums get
        # their own ones-matmul accumulation chain into a separate PSUM bank
        # (base partition 0, as the custom-DVE reciprocal requires).
        v_loc = pconst.tile([128, 2, H, 64], f8, name="v_loc")

        # tiles for the proj/fc1 weights; their DMAs are issued after the
        # AllGather triggers so they don't compete with the qkv weight loads
        wproj_sb = pconst.tile([128, 8, C], f8, name="wproj_sb")
        w1_sb = pconst.tile([128, 8, HID], bf16, name="w1_sb")

        x1_sb = pconst.tile([128, 2, C], f32, name="x1_sb")
        xb_sb = pconst.tile([128, 2, C], f32, name="xb_sb")
        qnT = pconst.tile([128, 8, TLOC], bf16, name="qnT")
        knT_loc = pconst.tile([128, 8, TLOC], f8, name="knT_loc")
        OT_sb = pconst.tile([128, 8, TLOC], bf16, name="OT_sb")

        bounce_kn = pdram.tile([128, 2048], f8, name="bounce_kn")
        ag_kn = pdram.tile([512, 2048], f8, name="ag_kn")
        bounce_v = pdram.tile([128, 2048], f8, name="bounce_v")
        ag_v = pdram.tile([512, 2048], f8, name="ag_v")

        def layernorm(pool, x_slice, out_t):
            # plain LN (affine is folded into the next matmul's weights)
            stats = pool.tile([128, 2, 6], f32, name="lnstats", tag="lnstats")
            for sg in range(2):
                nc.vector.bn_stats(out=stats[:, sg, :],
                                   in_=x_slice[:, sg * 512:(sg + 1) * 512])
            mv = pool.tile([128, 2], f32, name="lnmv", tag="lnmv")
            nc.vector.bn_aggr(out=mv, in_=stats)
            std = pool.tile([128, 1], f32, name="lnstd", tag="lnstd")
            nc.scalar.activation(out=std, in_=mv[:, 1:2], func=AF.Sqrt,
                                 bias=eps_sb[:, 0:1])
            rstd = pool.tile([128, 1], f32, name="lnrstd", tag="lnrstd")
            nc.vector.reciprocal_approx_fast(out=rstd, in_=std)
            nc.vector.tensor_scalar(out=out_t, in0=x_slice, scalar1=mv[:, 0:1],
                                    scalar2=rstd, op0=OP.subtract, op1=OP.mult)

        # ============== Phase A: LN1, qkv, norms, AllGathers ==============
        with tc.tile_pool(name="pA", bufs=1) as pA, \
             tc.tile_pool(name="psA", bufs=1, space="PSUM") as psA, \
             tc.tile_pool(name="ptmpA", bufs=2) as ptmpA:
            x_sb = pA.tile([128, 2, C], f32, name="x_sb")
            for tt in range(2):
                nc.sync.dma_start(x_sb[:, tt, :], x_in[tt * 128:(tt + 1) * 128, :])
            wqkv_sb = pA.tile([128, 8, 3 * C], f8, name="wqkv_sb")
            for cc in range(8):
                nc.sync.dma_start(wqkv_sb[:, cc, :], wqkv_in[cc])

            h_sb = pA.tile([128, 2, C], bf16, name="h_sb")
            for tt in range(2):
                layernorm(ptmpA, x_sb[:, tt, :], h_sb[:, tt, :])
            hT = pA.tile([128, 8, TLOC], bf16, name="hT")
            for tt in range(2):
                for cp in range(4):
                    tp = psA.tile([128, 2, 128], bf16, name="tp", tag="tp", bufs=2)
                    for k in range(2):
                        cc = 2 * cp + k
                        nc.tensor.transpose(
                            tp[:, k, :], h_sb[:, tt, cc * 128:(cc + 1) * 128], ident)
                    nc.scalar.activation(
                        out=hT[:, 2 * cp:2 * cp + 2, tt * 128:(tt + 1) * 128],
                        in_=tp, func=AF.Copy)

            qkv_sb = pA.tile([128, 24, TLOC], bf16, name="qkv_sb")

            def qkv_super(sup, evac_dve):
                # blk-outer so the two groups sharing a PSUM bank are
                # strictly sequential (start=True clears the whole bank)
                pss = [psA.tile([128, 2, TLOC], f32, name=f"qps{g}", tag="mm",
                                bufs=4) for g in range(2)]
                for blk in range(4):
                    for cc in range(8):
                        nc.tensor.matmul(
                            pss[blk // 2][:, blk % 2, :],
                            lhsT=wqkv_sb[:, cc,
                                         sup * 512 + blk * 128:
                                         sup * 512 + (blk + 1) * 128],
                            rhs=hT[:, cc, :],
                            start=(cc == 0), stop=(cc == 7))
                for blk in range(4):
                    cb = sup * 4 + blk
                    if evac_dve:
                        nc.vector.tensor_scalar(
                            out=qkv_sb[:, cb, :], in0=pss[blk // 2][:, blk % 2, :],
                            scalar1=RSCALE, scalar2=bqkv_sb[:, cb:cb + 1],
                            op0=OP.mult, op1=OP.add)
                    else:
                        nc.scalar.activation(
                            out=qkv_sb[:, cb, :], in_=pss[blk // 2][:, blk % 2, :],
                            func=AF.Identity, bias=bqkv_sb[:, cb:cb + 1],
                            scale=RSCALE)

            def norm_heads(pool, src_col0, dst, with_scale):
                q2 = pool.tile([128, 8, TLOC], bf16, name="q2", tag="q2", bufs=1)
                nc.vector.tensor_mul(q2, qkv_sb[:, src_col0:src_col0 + 8, :],
                                     qkv_sb[:, src_col0:src_col0 + 8, :])
                nrm = pool.tile([2, 8, TLOC], f32, name="nrm", tag="nrm0",
                                bufs=1)
                for g in range(4):
                    ssq = psA.tile([2, 2, TLOC], f32, name="ssq", tag="nrm",
                                   bufs=2)
                    for k in range(2):
                        nc.tensor.matmul(ssq[:, k, :], lhsT=sel_64,
                                         rhs=q2[:, 2 * g + k, :])
                    nc.scalar.activation(out=nrm[:, 2 * g:2 * g + 2, :],
                                         in_=ssq, func=AF.Sqrt,
                                         bias=tiny_sb[0:2, 0:1])
                rn_all = pool.tile([2, 8, TLOC], f32, name="rn_all", tag="rn",
                                   bufs=1)
                rn_flat = rn_all.rearrange("p a b -> p (a b)")
                nc.vector.reciprocal_approx_fast(
                    out=rn_flat, in_=nrm.rearrange("p a b -> p (a b)"))
                if with_scale:
                    nc.vector.tensor_mul(rn_flat, rn_flat,
                                         scales_sb.rearrange("p a b -> p (a b)"))
                rnr = pool.tile([2, 8, TLOC], bf16, name="rnr", tag="rnr", bufs=1)
                nc.vector.tensor_copy(rnr.rearrange("p a b -> p (a b)"), rn_flat)
                for blk in range(8):
                    bc = psA.tile([128, TLOC], f32, name="bc", tag="nrm", bufs=2)
                    nc.tensor.matmul(bc, lhsT=sel2T, rhs=rnr[:, blk, :])
                    nc.vector.tensor_mul(dst[:, blk, :], bc,
                                         qkv_sb[:, src_col0 + blk, :])

            # K first: the kn AllGather is the critical path of attention
            for sup in (2, 3):
                qkv_super(sup, evac_dve=False)
            norm_heads(ptmpA, 8, knT_loc, with_scale=False)
            nc.sync.dma_start(bounce_kn, knT_loc.rearrange("p a b -> p (a b)"))
            nc.gpsimd.collective_compute(
                "AllGather", OP.bypass,
                ins=[bounce_kn.opt()], outs=[ag_kn.opt()],
                replica_groups=[[0, 1, 2, 3], [4, 5, 6, 7]],
            )
            # V next, transposed into the per-head vext layout (+ones)
            for sup in (4, 5):
                qkv_super(sup, evac_dve=False)
            for tt in range(2):
                for cb in range(8):
                    tp2 = psA.tile([128, 128], bf16, name="tp2", tag="tp",
                                   bufs=2)
                    nc.tensor.transpose(tp2, qkv_sb[:, 16 + cb,
                                                    tt * 128:(tt + 1) * 128],
                                        ident)
                    nc.vector.tensor_copy(
                        v_loc[:, tt, 2 * cb:2 * cb + 2, :],
                        tp2.rearrange("p (a b) -> p a b", a=2))
            nc.sync.dma_start(bounce_v, v_loc.rearrange("p a b c -> p (a b c)"))
            nc.gpsimd.collective_compute(
                "AllGather", OP.bypass,
                ins=[bounce_v.opt()], outs=[ag_v.opt()],
                replica_groups=[[0, 1, 2, 3], [4, 5, 6, 7]],
            )
            nc.sync.dma_start(wproj_sb, wproj_in.rearrange("a p b -> p a b"))
            nc.sync.dma_start(w1_sb, w1_in.rearrange("a p b -> p a b"))
            # Q supers + q normalization overlap the collectives
            for sup in (0, 1):
                qkv_super(sup, evac_dve=True)
            norm_heads(ptmpA, 0, qnT, with_scale=True)
            # attn residual base: x + bproj (x_sb dies with phase A)
            for tt in range(2):
                nc.vector.tensor_add(xb_sb[:, tt, :], x_sb[:, tt, :], bpbc_sb)
            if DEBUG:
                nc.sync.dma_start(dbg_qkv[:], qkv_sb.rearrange("p a b -> p (a b)"))
                nc.sync.dma_start(dbg_kn[:], knT_loc.rearrange("p a b -> p (a b)"))
                nc.sync.dma_start(dbg_qn[:], qnT.rearrange("p a b -> p (a b)"))

        # ============== Phase B: attention + proj ==============
        with tc.tile_pool(name="pB", bufs=1) as pB, \
             tc.tile_pool(name="psB", bufs=1, space="PSUM") as psB, \
             tc.tile_pool(name="alst", bufs=3) as alst, \
             tc.tile_pool(name="pP", bufs=9) as pP, \
             tc.tile_pool(name="prs", bufs=2) as prs:
            kn_r, v_r = [], []
            for r in range(4):
                t = pB.tile([128, 2048], f8, name=f"kn{r}", tag=f"kn{r}")
                nc.sync.dma_start(t, ag_kn[r * 128:(r + 1) * 128, :])
                kn_r.append(t)
            for r in range(4):
                t = pB.tile([128, 2, H, 64], f8, name=f"v{r}", tag=f"v{r}")
                nc.sync.dma_start(t.rearrange("p a b c -> p (a b c)"),
                                  ag_v[r * 128:(r + 1) * 128, :])
                v_r.append(t)
            PLAG = 8  # heads of S/exp lead before each PV (hides AG2 latency)
            P_tiles = [None] * H
            rows_of = lambda h: slice(64 * (h % 2), 64 * (h % 2) + 64)

            def pass1_head(h):
                rows = rows_of(h)
                P = pP.tile([128, 8, TLOC], bf16, name="P", tag="P")
                P_tiles[h] = P
                for g in range(2):
                    al = alst.tile([128, 4, TLOC], bf16, name="al", tag="al")
                    nc.sync.dma_start(
                        al, alibi_in[h, 4 * g:4 * g + 4].rearrange(
                            "a p b -> p a b"))
                    S = psB.tile([128, 4, TLOC], f32, name="S", tag="s4", bufs=2)
                    for j in range(4):
                        b = 4 * g + j
                        r, tt = b // 2, b % 2
                        knT_sl = kn_r[r][rows, (h // 2) * 256 + tt * 128:
                                         (h // 2) * 256 + tt * 128 + 128]
                        nc.tensor.matmul(S[:, j, :], lhsT=knT_sl,
                                         rhs=qnT[rows, h // 2, :],
                                         start=True, stop=False)
                        nc.tensor.matmul(S[:, j, :], lhsT=ident, rhs=al[:, j, :],
                                         start=False, stop=True)
                    nc.scalar.activation(out=P[:, 4 * g:4 * g + 4, :], in_=S,
                                         func=AF.Exp,
                                         bias=nbound_sb[:, h:h + 1], scale=1.0)

            def pass2_head(h):
                rows = rows_of(h)
                P = P_tiles[h]
                pv_o = psB.tile([64, TLOC], f32, name="pv_o", tag="pvo", bufs=2)
                pv_s = psB.tile([64, TLOC], f32, name="pv_s", tag="pvs", bufs=2)
                for b in range(8):
                    r, tt = b // 2, b % 2
                    nc.tensor.matmul(pv_o, lhsT=v_r[r][:, tt, h, :],
                                     rhs=P[:, b, :],
                                     start=(b == 0), stop=(b == 7))
                for b in range(8):
                    nc.tensor.matmul(pv_s, lhsT=ones64, rhs=P[:, b, :],
                                     start=(b == 0), stop=(b == 7))
                rs = prs.tile([64, TLOC], f32, name="rs", tag="rs")
                nc.vector.reciprocal_approx_fast(out=rs, in_=pv_s)
                nc.vector.tensor_mul(OT_sb[rows, h // 2, :], pv_o, rs)
                if DEBUG and h == 0:
                    nc.sync.dma_start(dbg_rs[:], rs)
                    nc.sync.dma_start(dbg_P[:, 0:2048],
                                      P.rearrange("p a b -> p (a b)"))
                if DEBUG and h == 1:
                    nc.sync.dma_start(dbg_P[:, 2048:4096],
                                      P.rearrange("p a b -> p (a b)"))
                P_tiles[h] = None

            for h in range(H):
                pass1_head(h)
                if h >= PLAG:
                    pass2_head(h - PLAG)
            for h in range(H - PLAG, H):
                pass2_head(h)

            if DEBUG:
                nc.sync.dma_start(dbg_OT[:], OT_sb.rearrange("p a b -> p (a b)"))
                nc.sync.dma_start(dbg_vr0[:],
                                  v_r[0].rearrange("p a b c -> p (a b c)"))
            # proj, token-major: x1 = O @ Wproj / WSCALE + (x + bproj)
            for th in range(2):
                for ch in range(2):
                    pp = psB.tile([128, 512], f32, name="pp", tag="s4", bufs=2)
                    for cc in range(8):
                        nc.tensor.matmul(
                            pp, lhsT=OT_sb[:, cc, th * 128:(th + 1) * 128],
                            rhs=wproj_sb[:, cc, ch * 512:(ch + 1) * 512],
                            start=(cc == 0), stop=(cc == 7))
                    nc.vector.scalar_tensor_tensor(
                        out=x1_sb[:, th, ch * 512:(ch + 1) * 512],
                        in0=pp, scalar=RSCALE,
                        in1=xb_sb[:, th, ch * 512:(ch + 1) * 512],
                        op0=OP.mult, op1=OP.add)

        if DEBUG:
            nc.sync.dma_start(dbg_x1[:], x1_sb.rearrange("p a b -> p (a b)"))

        # ================= Phase C: LN2 + MLP =================
        with tc.tile_pool(name="pC", bufs=1) as pC, \
             tc.tile_pool(name="psC", bufs=1, space="PSUM") as psC, \
             tc.tile_pool(name="wstC", bufs=8) as wstC, \
             tc.tile_pool(name="ptmpC", bufs=2) as ptmpC:
            out_sb = pC.tile([128, 2, C], f32, name="out_sb")
            y_sb = pC.tile([128, 2, C], bf16, name="y_sb")
            for tt in range(2):
                layernorm(ptmpC, x1_sb[:, tt, :], y_sb[:, tt, :])
            yT = pC.tile([128, 8, TLOC], bf16, name="yT")
            for tt in range(2):
                for cp in range(4):
                    tp4 = psC.tile([128, 2, 128], bf16, name="tp4", tag="tp",
                                   bufs=2)
                    for k in range(2):
                        cc = 2 * cp + k
                        nc.tensor.transpose(
                            tp4[:, k, :], y_sb[:, tt, cc * 128:(cc + 1) * 128],
                            ident)
                    nc.vector.tensor_copy(
                        yT[:, 2 * cp:2 * cp + 2, tt * 128:(tt + 1) * 128],
                        tp4)
            # mlp residual base: x1 + b2 (per-C broadcast)
            x1b_sb = pC.tile([128, 2, C], f32, name="x1b_sb")
            for tt in range(2):
                nc.vector.tensor_add(x1b_sb[:, tt, :], x1_sb[:, tt, :], b2bc_sb)

            h1 = pC.tile([128, 32, TLOC], bf16, name="h1")
            for sup in range(8):
                pss = [psC.tile([128, 2, TLOC], f32, name=f"m1ps{g}", tag="mm",
                                bufs=4) for g in range(2)]
                for blk in range(4):
                    for cc in range(8):
                        nc.tensor.matmul(
                            pss[blk // 2][:, blk % 2, :],
                            lhsT=w1_sb[:, cc,
                                       sup * 512 + blk * 128:
                                       sup * 512 + (blk + 1) * 128],
                            rhs=yT[:, cc, :],
                            start=(cc == 0), stop=(cc == 7))
                for blk in range(4):
                    hb = sup * 4 + blk
                    nc.scalar.activation(out=h1[:, hb, :],
                                         in_=pss[blk // 2][:, blk % 2, :],
                                         func=AF.Gelu,
                                         bias=b1_sb[:, hb:hb + 1], scale=1.0)

            # fc2, token-major, single pass over w2:
            # out = h1 @ W2 / WSCALE + (x1 + b2)
            pss2 = [psC.tile([128, 512], f32, name=f"m2ps{g}", tag="mm",
                             bufs=4) for g in range(4)]
            for hc in range(32):
                w2t = wstC.tile([128, C], f8e3, name="w2t", tag="w")
                nc.sync.dma_start(w2t, w2_in[hc])
                for th in range(2):
                    for ch in range(2):
                        nc.tensor.matmul(
                            pss2[th * 2 + ch],
                            lhsT=h1[:, hc, th * 128:(th + 1) * 128],
                            rhs=w2t[:, ch * 512:(ch + 1) * 512],
                            start=(hc == 0), stop=(hc == 31))
            for th in range(2):
                for ch in range(2):
                    nc.vector.scalar_tensor_tensor(
                        out=out_sb[:, th, ch * 512:(ch + 1) * 512],
                        in0=pss2[th * 2 + ch], scalar=1.0 / 128.0,
                        in1=x1b_sb[:, th, ch * 512:(ch + 1) * 512],
                        op0=OP.mult, op1=OP.add)
            for tt in range(2):
                nc.sync.dma_start(out_ext[tt * 128:(tt + 1) * 128, :],
                                  out_sb[:, tt, :])

    nc.finalize()
    return nc


def _get_nc():
    if "nc" not in _CACHE:
        _CACHE["nc"] = _build_nc()
    return _CACHE["nc"]


def _to_f8(w):
    import ml_dtypes
    return np.clip(w, -240.0, 240.0).astype(ml_dtypes.float8_e4m3fn)


def _make_in_maps(inputs):
    import ml_dtypes
    bf = ml_dtypes.bfloat16
    x = np.asarray(inputs["x"], np.float32)
    mask = np.asarray(inputs["padding_mask"]).astype(bool)
    alibi = np.asarray(inputs["alibi_bias"], np.float32)
    wqkv = np.asarray(inputs["Wqkv"], np.float32)
    bqkv = np.asarray(inputs["bqkv"], np.float32)
    wproj = np.asarray(inputs["Wproj"], np.float32)
    bproj = np.asarray(inputs["bproj"], np.float32)
    w1 = np.asarray(inputs["W1"], np.float32)
    b1 = np.asarray(inputs["b1"], np.float32)
    w2 = np.asarray(inputs["W2"], np.float32)
    b2 = np.asarray(inputs["b2"], np.float32)
    g1 = np.asarray(inputs["ln1_g"], np.float32)
    bln1 = np.asarray(inputs["ln1_b"], np.float32)
    g2 = np.asarray(inputs["ln2_g"], np.float32)
    bln2 = np.asarray(inputs["ln2_b"], np.float32)
    ls = np.asarray(inputs["logit_scale"], np.float32).reshape(H)
    scale = np.exp(np.minimum(ls, math.log(100.0))).astype(np.float32)
    amax = float(alibi.max())
    bound = scale + amax + 1.0
    nbound = np.ascontiguousarray(np.tile((-bound).astype(np.float32)[None, :],
                                          (128, 1)))

    # fold LN affine into the consuming weight matrices
    wqkv_eff = g1[:, None] * wqkv
    bqkv_eff = bqkv + bln1 @ wqkv
    w1_eff = g2[:, None] * w1
    b1_eff = b1 + bln2 @ w1

    consts = np.zeros((128, 384), dtype=np.float32)
    consts[:, 0:128] = np.eye(128, dtype=np.float32)
    consts[:, 128:192] = 1.0
    consts[0:64, 192] = 1.0
    consts[64:128, 193] = 1.0
    consts[0, 194:258] = 1.0
    consts[1, 258:322] = 1.0
    consts = np.ascontiguousarray(consts)
    scales_bc = np.zeros((2, 8, 256), dtype=np.float32)
    for h in range(H):
        scales_bc[h % 2, h // 2, :] = scale[h]

    common = {
        "wqkv_t": _to_f8((WSCALE * wqkv_eff).reshape(8, 128, 3 * C)),
        "bqkv_t": np.ascontiguousarray(bqkv_eff.reshape(24, 128).T),
        "wproj_t": _to_f8((WSCALE * wproj).reshape(8, 128, C)),
        "w1_t": np.ascontiguousarray(w1_eff.reshape(8, 128, HID)).astype(bf),
        "b1_t": np.ascontiguousarray(b1_eff.reshape(32, 128).T),
        "w2_t": np.clip(128.0 * w2.reshape(32, 128, C), -15.5, 15.5).astype(ml_dtypes.float8_e3m4),
        "bprow": np.ascontiguousarray(bproj.reshape(1, C)),
        "b2row": np.ascontiguousarray(b2.reshape(1, C)),
        "scales": scales_bc.astype(bf),
        "nbound": nbound,
        "consts_bf": consts.astype(bf),
    }
    in_maps = []
    for c in range(NCORES):
        b, qi = divmod(c, GROUP)
        q0 = qi * TLOC
        alT = alibi[b, :, q0:q0 + TLOC, :].transpose(0, 2, 1)  # [H, N(k), TLOC]
        alT = alT + np.where(mask[b], np.float32(-1e9),
                             np.float32(0.0)).astype(np.float32)[None, :, None]
        alT = np.ascontiguousarray(
            alT.reshape(H, 8, 128, TLOC)).astype(bf)
        m = dict(common)
        m["x_loc"] = np.ascontiguousarray(x[b, q0:q0 + TLOC, :])
        m["alibi_t"] = alT
        in_maps.append(m)
    return in_maps


def _run(inputs, trace=False):
    from concourse import bass_utils
    nc = _get_nc()
    in_maps = _make_in_maps(inputs)
    res = bass_utils.run_bass_kernel_spmd(
        nc, in_maps, core_ids=list(range(NCORES)), trace=trace)
    outs = [np.asarray(res.results[c]["out"]) for c in range(NCORES)]
    y = np.stack(outs).reshape(B, GROUP * TLOC, C)
    return y.astype(np.float32), res


def kernel(**inputs):
    y, _ = _run(inputs, trace=False)
    return y
